# revision 20
# baseline (speedup 1.0000x reference)
"""Trainium2 Bass kernel for nn_MCLMask (bipartite Katz / MCL mask).

Math (derived from the reference):
  S[bq][m,s] = cosine sim of query pos m vs class-proto pos s  (m=100, s=500)
  T_sq = softmax(20 S, over s); T_qs = softmax(10 S^T, over m)
  T = [[0, T_sq^T],[T_qs^T, 0]];  katz = solve(I-0.5T, 1) - 1
  out = normalize(katz[500:])
Bipartite reduction: only xq is needed:
  (I - 0.25 C B) xq = 1 + 0.5 C 1,  with B=T_sq^T (500x100), C=T_qs^T (100x500).
With E10=exp(10*Shat), E20=E10^2, cs[s]=sum_m E10[m,s], rs[m']=sum_s E20[m',s]:
  C = F^T where F[s,m] = E10[s,m]/cs[s];  B[s,m'] = E20[s,m']/rs[m']
  Chat^T = E20^T F (per pair, contraction over s), CB = Chat diag(1/rs)
  Solve via Neumann iteration in z = 0.25 D x space (|0.25 CB|_1 = 0.25).

Sharding: core = 2*b + half; each core owns 38 (b,q) pairs (q padded 75->76).
Device layout: s on partitions (4 tiles of 125), (pair,m) on free (3800).
"""
import sys
import os
import numpy as np

for _p in ("/opt/trn_rl_repo",):
    if os.path.isdir(_p) and _p not in sys.path:
        sys.path.insert(0, _p)

import concourse.bass as bass
import concourse.bacc as bacc
import concourse.mybir as mybir
import concourse.tile as tile
from concourse.bass_utils import run_bass_kernel_spmd

F32 = mybir.dt.float32
F32R = mybir.dt.float32r
BF16 = mybir.dt.bfloat16
AX = mybir.AxisListType
OP = mybir.AluOpType
ACTF = mybir.ActivationFunctionType

# problem constants (hardcoded per contract)
B_, S_, C_, HW = 4, 25, 640, 100
NW, KS = 5, 5
Q_ = 75
PP = 38            # (b,q) pairs per core
M = 100            # query spatial positions per pair
MS = 500           # support positions (n_way * HW)
CCH = 5            # c chunks of 128
STIL = 4           # s tiles of 125
SP = 125           # partitions per s tile
NCH = 8            # free chunks of 475 over 3800
NF = PP * M        # 3800
FW = 101           # F segment width (100 + ones col)
NFF = PP * FW      # 3838
CTW = 128          # padded CT segment width
NITER = 3          # Neumann applications (2 chained + final)

_CACHED = {}


def build_nc():
    nc = bacc.Bacc("TRN2", target_bir_lowering=False, debug=False)
    d_sup = nc.declare_dram_parameter("sup", [C_, S_ * HW], F32, isOutput=False)
    d_qry = nc.declare_dram_parameter("qry", [C_, NF], F32, isOutput=False)
    d_out = nc.declare_dram_parameter("out", [PP, M], F32, isOutput=True)
    # DRAM scratch for row<->col transposes
    d_snr = nc.dram_tensor("d_snr", [1, MS], F32)
    d_ar = nc.dram_tensor("d_ar", [1, NF], F32)
    d_fs = nc.dram_tensor("d_fs", [1, NFF], F32)
    d_kq = nc.dram_tensor("d_kq", [PP, M], F32)

    with tile.TileContext(nc) as tc:
        from contextlib import ExitStack
        with ExitStack() as ctx:
            ek = ctx.enter_context
            p_const = ek(tc.tile_pool(name="const", bufs=1))
            p_sup = ek(tc.tile_pool(name="sup", bufs=1))
            p_qry = ek(tc.tile_pool(name="qry", bufs=1))
            p_small = ek(tc.tile_pool(name="small", bufs=1))
            p_ps = ek(tc.tile_pool(name="ps", bufs=8, space="PSUM"))

            ones128b = p_const.tile([128, 1], BF16)
            nc.vector.memset(ones128b[:], 1.0)
            ones125b = p_const.tile([SP, 1], BF16)
            nc.vector.memset(ones125b[:], 1.0)

            # ---- phase 1a: support load, shot-mean (bf16), sup norm ----
            sup_mean = []      # per c-chunk (128, 500) bf16
            qry_t = []         # per c-chunk (128, 3800) bf16
            ph1 = ExitStack()
            p_supraw = ph1.enter_context(tc.tile_pool(name="supraw", bufs=2))
            p_qryf = ph1.enter_context(tc.tile_pool(name="qryf", bufs=2))
            p_sq = ph1.enter_context(tc.tile_pool(name="sq", bufs=1))
            ns2s_ps = p_ps.tile([1, MS], F32, tag="ps", name="ns2s")
            for k in range(CCH):
                sraw = p_supraw.tile([128, S_ * HW], F32, tag="sraw")
                nc.sync.dma_start(sraw[:], d_sup[:][128 * k:128 * (k + 1), :])
                sm = p_sup.tile([128, MS], BF16, tag=f"supm{k}")
                with nc.allow_low_precision(reason="bf16 shot-mean validated"):
                    nc.vector.tensor_reduce(
                        sm[:], sraw[:].rearrange("p (n m k) -> p n m k", n=NW, k=KS, m=HW),
                        axis=AX.X, op=OP.add)
                sup_mean.append(sm)
                sqs = p_sq.tile([128, MS], BF16, tag="sqs")
                nc.vector.tensor_tensor(sqs[:], sm[:], sm[:], op=OP.mult)
                nc.tensor.matmul(ns2s_ps[:], ones128b[:], sqs[:],
                                 start=(k == 0), stop=(k == CCH - 1))
            # ---- phase 1b: query load, bf16 cast, qry norm ----
            ns2q_ps = [p_ps.tile([1, 475], F32, tag="ps", name=f"ns2q{_n}") for _n in range(NCH)]
            for k in range(CCH):
                qf = p_qryf.tile([128, NF], F32, tag="qryf")
                nc.sync.dma_start(qf[:], d_qry[:][128 * k:128 * (k + 1), :])
                qt = p_qry.tile([128, NF], BF16, tag=f"qry{k}")
                nc.scalar.activation(qt[:], qf[:], ACTF.Copy)
                qry_t.append(qt)
                sqq = p_sq.tile([128, NF], BF16, tag="sqq")
                nc.vector.tensor_tensor(sqq[:], qt[:], qt[:], op=OP.mult)
                for n in range(NCH):
                    nc.tensor.matmul(ns2q_ps[n][:], ones128b[:],
                                     sqq[:][:, 475 * n:475 * (n + 1)],
                                     start=(k == 0), stop=(k == CCH - 1))

            ph1.close()
            p_rows = ek(tc.tile_pool(name="rows", bufs=1))
            p_abc = ek(tc.tile_pool(name="abc", bufs=1))
            p_e10 = ek(tc.tile_pool(name="e10", bufs=2))
            p_big = ek(tc.tile_pool(name="big", bufs=1))

            # ---- phase 2: norms -> scale rows / cols ----
            # snr10_row = 10 / sqrt(ns2_sup); via 0.1*sqrt then recip
            nrm_s = p_small.tile([1, MS], F32)
            nc.scalar.activation(nrm_s[:], ns2s_ps[:], ACTF.Sqrt, scale=0.01)
            # sqrt(0.01*x) = 0.1*sqrt(x); recip -> 10/sqrt(x)
            snr10 = p_small.tile([1, MS], F32)
            nc.vector.reciprocal_approx_fast(snr10[:], nrm_s[:])
            nc.sync.dma_start(d_snr[:], snr10[:])
            snr_col = []
            for j in range(STIL):
                scol = p_small.tile([SP, 1], F32, tag=f"snrc{j}")
                nc.sync.dma_start(scol[:], d_snr[:][:, SP * j:SP * (j + 1)].transpose([1, 0]))
                snr_col.append(scol)
            # a_row = 1/sqrt(ns2_q); rowbuf reused later for fs_row
            rowbuf = p_rows.tile([1, NFF], F32, tag="rowbuf", name="rowq")
            nrm_q = rowbuf[:][:, 0:NF]
            for n in range(NCH):
                nc.scalar.activation(nrm_q[:, 475 * n:475 * (n + 1)], ns2q_ps[n][:],
                                     ACTF.Sqrt)
            nc.vector.reciprocal_approx_fast(nrm_q, nrm_q)
            nc.sync.dma_start(d_ar[:], nrm_q)
            a_bc = p_abc.tile([128, NF], F32)
            nc.sync.dma_start(a_bc[:], d_ar[:].broadcast_to((128, NF)))

            # ---- phase 3: S_T matmul, exp, E20, cs, F ----
            E20 = []
            Ft = []
            for j in range(STIL):
                e10 = p_e10.tile([SP, NF], BF16, tag="e10")
                for n in range(NCH):
                    st_ps = p_ps.tile([SP, 475], F32, tag="ps", name=f"st{j}_{n}")
                    for k in range(CCH):
                        nc.tensor.matmul(st_ps[:],
                                         sup_mean[k][:][:, SP * j:SP * (j + 1)],
                                         qry_t[k][:][:, 475 * n:475 * (n + 1)],
                                         start=(k == 0), stop=(k == CCH - 1))
                    # in-psum query-norm scaling, then exp with per-partition sup scale
                    nc.vector.tensor_tensor(
                        st_ps[:], st_ps[:],
                        a_bc[:][0:SP, 475 * n:475 * (n + 1)], op=OP.mult)
                    nc.scalar.activation(
                        e10[:][:, 475 * n:475 * (n + 1)], st_ps[:],
                        ACTF.Exp, scale=snr_col[j][:])
                e20 = p_big.tile([SP, NF], BF16, tag=f"e20_{j}")
                nc.scalar.activation(e20[:], e10[:], ACTF.Square)
                E20.append(e20)
                cs = p_small.tile([SP, PP], F32, tag=f"cs{j}")
                nc.vector.tensor_reduce(
                    cs[:], e10[:].rearrange("p (q m) -> p q m", m=M),
                    axis=AX.X, op=OP.add)
                csr = p_small.tile([SP, PP], F32, tag=f"csr{j}")
                nc.vector.reciprocal_approx_fast(csr[:], cs[:])
                ft = p_big.tile([SP, NFF], BF16, tag=f"f_{j}")
                nc.vector.memset(
                    ft[:].rearrange("p (q r) -> p q r", r=FW)[:, :, M:FW], 1.0)
                nc.vector.tensor_tensor(
                    ft[:].rearrange("p (q r) -> p q r", r=FW)[:, :, 0:M],
                    e10[:].rearrange("p (q m) -> p q m", m=M),
                    csr[:].unsqueeze(2).broadcast_to((SP, PP, M)),
                    op=OP.mult)
                Ft.append(ft)

            # ---- phase 4: per-pair Chat^T (+rs col) matmuls ----
            ctpad = p_big.tile([M, PP * CTW], BF16)
            nc.vector.memset(
                ctpad[:].rearrange("p (q r) -> p q r", r=CTW)[:, :, M:CTW], 0.0)
            rs_all = p_small.tile([M, PP], F32)
            for p in range(PP):
                ct_ps = p_ps.tile([M, FW], F32, tag="ps")
                for j in range(STIL):
                    nc.tensor.matmul(ct_ps[:],
                                     E20[j][:][:, M * p:M * (p + 1)],
                                     Ft[j][:][:, FW * p:FW * (p + 1)],
                                     start=(j == 0), stop=(j == STIL - 1))
                nc.vector.tensor_copy(ctpad[:][:, CTW * p:CTW * p + M], ct_ps[:][:, 0:M])
                nc.vector.tensor_copy(rs_all[:][:, p:p + 1], ct_ps[:][:, M:FW])

            # ---- phase 4b: FS row sums (ones-matmuls over F) + transpose ----
            fs_chunks = [(i * FW * 5, min(FW * 5, NFF - i * FW * 5)) for i in range(8)]
            fs_ps = [p_ps.tile([1, w], F32, tag="ps", name=f"fsps{_i}") for _i, (_, w) in enumerate(fs_chunks)]
            for j in range(STIL):
                for i, (off, w) in enumerate(fs_chunks):
                    nc.tensor.matmul(fs_ps[i][:], ones125b[:],
                                     Ft[j][:][:, off:off + w],
                                     start=(j == 0), stop=(j == STIL - 1))
            fs_row = p_rows.tile([1, NFF], F32, tag="rowbuf", name="rowfs")
            for i, (off, w) in enumerate(fs_chunks):
                nc.scalar.activation(fs_row[:][:, off:off + w], fs_ps[i][:], ACTF.Copy)
            nc.sync.dma_start(d_fs[:], fs_row[:])
            fs_col = p_small.tile([M, PP], F32)
            nc.sync.dma_start(
                fs_col[:],
                d_fs[:].rearrange("o (q r) -> r q", r=FW)[0:M, :])

            # ---- phase 5: solve (Neumann in z space) ----
            rsr = p_small.tile([M, PP], F32)
            nc.vector.reciprocal_approx_fast(rsr[:], rs_all[:])
            qrsr = p_small.tile([M, PP], F32)
            nc.vector.tensor_scalar(qrsr[:], rsr[:], 0.25, None, op0=OP.mult)
            rhsv = p_small.tile([M, PP], F32)
            nc.vector.tensor_scalar(rhsv[:], fs_col[:], 0.5, 1.0, op0=OP.mult, op1=OP.add)
            qdr = p_small.tile([M, PP], F32)
            nc.vector.tensor_tensor(qdr[:], qrsr[:], rhsv[:], op=OP.mult)
            halffs = p_small.tile([M, PP], F32)
            nc.vector.tensor_scalar(halffs[:], fs_col[:], 0.5, None, op0=OP.mult)
            zt = p_small.tile([M, PP], BF16)
            nc.vector.tensor_copy(zt[:], qdr[:])
            kq_col = p_small.tile([M, PP], F32)
            wtmp = p_small.tile([M, PP], F32)
            for it in range(NITER):
                last = (it == NITER - 1)
                w_ps = p_ps.tile([CTW, PP], F32, tag="ps", name=f"w{it}")
                for p in range(PP):
                    nc.tensor.matmul(w_ps[:][:, p:p + 1],
                                     ctpad[:][:, CTW * p:CTW * (p + 1)],
                                     zt[:][:, p:p + 1], start=True, stop=True)
                if not last:
                    nc.vector.tensor_tensor(wtmp[:], w_ps[:][0:M, :], qrsr[:],
                                            op=OP.mult)
                    nc.vector.tensor_tensor(zt[:], wtmp[:], qdr[:], op=OP.add)
                else:
                    nc.vector.tensor_tensor(kq_col[:], w_ps[:][0:M, :], halffs[:],
                                            op=OP.add)

            # ---- phase 6: sums via PE ones-matvec, row-space normalize ----
            kq_bf = p_small.tile([M, PP], BF16)
            nc.vector.tensor_copy(kq_bf[:], kq_col[:])
            ssum_ps = p_ps.tile([PP, 1], F32, tag="ps", name="ssum")
            nc.tensor.matmul(ssum_ps[:], kq_bf[:], ones125b[:][0:M, :],
                             start=True, stop=True)
            sinv = p_small.tile([PP, 1], F32)
            nc.vector.reciprocal_approx_fast(sinv[:], ssum_ps[:])
            nc.sync.dma_start(d_kq[:].transpose([1, 0]), kq_col[:])
            kq_row = p_small.tile([PP, M], F32)
            nc.sync.dma_start(kq_row[:], d_kq[:])
            out_t = p_small.tile([PP, M], F32)
            nc.vector.tensor_scalar(out_t[:], kq_row[:], sinv[:], None, op0=OP.mult)
            nc.sync.dma_start(d_out[:], out_t[:])

    nc.compile()
    return nc


def shard_inputs(support_xf, query_xf):
    support_xf = np.asarray(support_xf, dtype=np.float32)
    query_xf = np.asarray(query_xf, dtype=np.float32)
    in_maps = []
    for core in range(8):
        b = core // 2
        half = core % 2
        qs = np.clip(np.arange(half * PP, half * PP + PP), 0, Q_ - 1)
        sup = np.ascontiguousarray(
            support_xf[b].reshape(NW, KS, C_, HW).transpose(2, 0, 3, 1).reshape(C_, S_ * HW))
        qry = np.ascontiguousarray(
            query_xf[b, qs].reshape(PP, C_, HW).transpose(1, 0, 2).reshape(C_, NF))
        in_maps.append({"sup": sup, "qry": qry})
    return in_maps


def run_sharded(support_xf, query_xf, trace=False, **kw):
    if "nc" not in _CACHED:
        _CACHED["nc"] = build_nc()
    nc = _CACHED["nc"]
    in_maps = shard_inputs(support_xf, query_xf)
    res = run_bass_kernel_spmd(nc, in_maps, core_ids=list(range(8)), trace=trace, **kw)
    b, q = np.asarray(support_xf).shape[0], np.asarray(query_xf).shape[1]
    out = np.zeros((b, q, 1, 10, 10), np.float32)
    for core in range(8):
        bi = core // 2
        half = core % 2
        real = min(PP, q - half * PP)
        o = res.results[core]["out"][:real]
        out[bi, half * PP: half * PP + real] = o.reshape(real, 1, 10, 10)
    return out, res


def kernel(support_xf, query_xf, n_way=5, k_shot=5):
    out, _ = run_sharded(support_xf, query_xf, trace=False)
    return out


# revision 21
# speedup vs baseline: 1.1326x; 1.1326x over previous
"""Trainium2 Bass kernel for nn_MCLMask (bipartite Katz / MCL mask).

Math (derived from the reference):
  S[bq][m,s] = cosine sim of query pos m vs class-proto pos s  (m=100, s=500)
  T_sq = softmax(20 S, over s); T_qs = softmax(10 S^T, over m)
  T = [[0, T_sq^T],[T_qs^T, 0]];  katz = solve(I-0.5T, 1) - 1
  out = normalize(katz[500:])
Bipartite reduction: only xq is needed:
  (I - 0.25 C B) xq = 1 + 0.5 C 1,  with B=T_sq^T (500x100), C=T_qs^T (100x500).
With E10=exp(10*Shat), E20=E10^2, cs[s]=sum_m E10[m,s], rs[m']=sum_s E20[m',s]:
  C = F^T where F[s,m] = E10[s,m]/cs[s];  B[s,m'] = E20[s,m']/rs[m']
  Chat^T = E20^T F (per pair, contraction over s), CB = Chat diag(1/rs)
  Solve via Neumann iteration in z = 0.25 D x space (|0.25 CB|_1 = 0.25).

Sharding: core = 2*b + half; each core owns 38 (b,q) pairs (q padded 75->76).
Device layout: s on partitions (4 tiles of 125), (pair,m) on free (3800).
"""
import sys
import os
import numpy as np

for _p in ("/opt/trn_rl_repo",):
    if os.path.isdir(_p) and _p not in sys.path:
        sys.path.insert(0, _p)

import concourse.bass as bass
import concourse.bacc as bacc
import concourse.mybir as mybir
import concourse.tile as tile
from concourse.bass_utils import run_bass_kernel_spmd

F32 = mybir.dt.float32
F32R = mybir.dt.float32r
BF16 = mybir.dt.bfloat16
AX = mybir.AxisListType
OP = mybir.AluOpType
ACTF = mybir.ActivationFunctionType

# problem constants (hardcoded per contract)
B_, S_, C_, HW = 4, 25, 640, 100
NW, KS = 5, 5
Q_ = 75
PP = 38            # (b,q) pairs per core
M = 100            # query spatial positions per pair
MS = 500           # support positions (n_way * HW)
CCH = 5            # c chunks of 128
STIL = 4           # s tiles of 125
SP = 125           # partitions per s tile
NCH = 8            # free chunks of 475 over 3800
NF = PP * M        # 3800
FW = 101           # F segment width (100 + ones col)
NFF = PP * FW      # 3838
CTW = 128          # padded CT segment width
NITER = 3          # Neumann applications (2 chained + final)

_CACHED = {}


def build_nc():
    nc = bacc.Bacc("TRN2", target_bir_lowering=False, debug=False)
    d_sup = nc.declare_dram_parameter("sup", [C_, S_ * HW], F32, isOutput=False)
    d_qry = nc.declare_dram_parameter("qry", [C_, NF], F32, isOutput=False)
    d_out = nc.declare_dram_parameter("out", [PP, M], F32, isOutput=True)
    # DRAM scratch for row<->col transposes
    d_snr = nc.dram_tensor("d_snr", [1, MS], F32)
    d_ar = nc.dram_tensor("d_ar", [1, NF], F32)
    d_fs = nc.dram_tensor("d_fs", [1, NFF], F32)
    d_kq = nc.dram_tensor("d_kq", [PP, M], F32)

    with tile.TileContext(nc) as tc:
        from contextlib import ExitStack
        with ExitStack() as ctx:
            ek = ctx.enter_context
            p_const = ek(tc.tile_pool(name="const", bufs=1))
            p_sup = ek(tc.tile_pool(name="sup", bufs=1))
            p_qry = ek(tc.tile_pool(name="qry", bufs=1))
            p_small = ek(tc.tile_pool(name="small", bufs=1))
            p_ps = ek(tc.tile_pool(name="ps", bufs=8, space="PSUM"))

            ones128b = p_const.tile([128, 1], BF16)
            nc.vector.memset(ones128b[:], 1.0)
            ones125b = p_const.tile([SP, 1], BF16)
            nc.vector.memset(ones125b[:], 1.0)

            # ---- phase 1a: support load, shot-mean (bf16), sup norm ----
            sup_mean = []      # per c-chunk (128, 500) bf16
            qry_t = []         # per c-chunk (128, 3800) bf16
            ph1 = ExitStack()
            p_supraw = ph1.enter_context(tc.tile_pool(name="supraw", bufs=2))
            p_qryf = ph1.enter_context(tc.tile_pool(name="qryf", bufs=2))
            p_sq = ph1.enter_context(tc.tile_pool(name="sq", bufs=1))
            ns2s_ps = p_ps.tile([1, MS], F32, tag="ps", name="ns2s")
            for k in range(CCH):
                sraw = p_supraw.tile([128, S_ * HW], F32, tag="sraw")
                nc.sync.dma_start(sraw[:], d_sup[:][128 * k:128 * (k + 1), :])
                sm = p_sup.tile([128, MS], BF16, tag=f"supm{k}")
                with nc.allow_low_precision(reason="bf16 shot-mean validated"):
                    nc.vector.tensor_reduce(
                        sm[:], sraw[:].rearrange("p (n m k) -> p n m k", n=NW, k=KS, m=HW),
                        axis=AX.X, op=OP.add)
                sup_mean.append(sm)
                sqs = p_sq.tile([128, MS], BF16, tag="sqs")
                nc.vector.tensor_tensor(sqs[:], sm[:], sm[:], op=OP.mult)
                nc.tensor.matmul(ns2s_ps[:], ones128b[:], sqs[:],
                                 start=(k == 0), stop=(k == CCH - 1))
            # ---- phase 1b: query load, bf16 cast, qry norm ----
            ns2q_ps = [p_ps.tile([1, 475], F32, tag="ps", name=f"ns2q{_n}") for _n in range(NCH)]
            for k in range(CCH):
                qf = p_qryf.tile([128, NF], F32, tag="qryf")
                nc.sync.dma_start(qf[:], d_qry[:][128 * k:128 * (k + 1), :])
                qt = p_qry.tile([128, NF], BF16, tag=f"qry{k}")
                nc.scalar.activation(qt[:], qf[:], ACTF.Copy)
                qry_t.append(qt)
                sqq = p_sq.tile([128, NF], BF16, tag="sqq")
                nc.vector.tensor_tensor(sqq[:], qt[:], qt[:], op=OP.mult)
                for n in range(NCH):
                    nc.tensor.matmul(ns2q_ps[n][:], ones128b[:],
                                     sqq[:][:, 475 * n:475 * (n + 1)],
                                     start=(k == 0), stop=(k == CCH - 1))

            ph1.close()
            p_rows = ek(tc.tile_pool(name="rows", bufs=1))
            p_abc = ek(tc.tile_pool(name="abc", bufs=1))
            p_e10 = ek(tc.tile_pool(name="e10", bufs=2))
            p_big = ek(tc.tile_pool(name="big", bufs=1))

            # ---- phase 2: norms -> scale rows / cols ----
            # snr10_row = 10 / sqrt(ns2_sup); via 0.1*sqrt then recip
            nrm_s = p_small.tile([1, MS], F32)
            nc.scalar.activation(nrm_s[:], ns2s_ps[:], ACTF.Sqrt, scale=0.01)
            # sqrt(0.01*x) = 0.1*sqrt(x); recip -> 10/sqrt(x)
            snr10 = p_small.tile([1, MS], F32)
            nc.vector.reciprocal_approx_fast(snr10[:], nrm_s[:])
            nc.sync.dma_start(d_snr[:], snr10[:])
            snr_col = []
            for j in range(STIL):
                scol = p_small.tile([SP, 1], F32, tag=f"snrc{j}")
                nc.sync.dma_start(scol[:], d_snr[:][:, SP * j:SP * (j + 1)].transpose([1, 0]))
                snr_col.append(scol)
            # a_row = 1/sqrt(ns2_q); rowbuf reused later for fs_row
            rowbuf = p_rows.tile([1, NFF], F32, tag="rowbuf", name="rowq")
            nrm_q = rowbuf[:][:, 0:NF]
            for n in range(NCH):
                nc.scalar.activation(nrm_q[:, 475 * n:475 * (n + 1)], ns2q_ps[n][:],
                                     ACTF.Sqrt)
            nc.vector.reciprocal_approx_fast(nrm_q, nrm_q)
            nc.sync.dma_start(d_ar[:], nrm_q)
            a_bc = p_abc.tile([128, NF], F32)
            nc.sync.dma_start(a_bc[:], d_ar[:].broadcast_to((128, NF)))

            # ---- phase 3: S_T matmul, exp, E20, cs, F ----
            E20 = []
            Ft = []
            for j in range(STIL):
                e10 = p_e10.tile([SP, NF], BF16, tag="e10")
                for n in range(NCH):
                    st_ps = p_ps.tile([SP, 475], F32, tag="ps", name=f"st{j}_{n}")
                    for k in range(CCH):
                        nc.tensor.matmul(st_ps[:],
                                         sup_mean[k][:][:, SP * j:SP * (j + 1)],
                                         qry_t[k][:][:, 475 * n:475 * (n + 1)],
                                         start=(k == 0), stop=(k == CCH - 1))
                    # in-psum query-norm scaling, then exp with per-partition sup scale
                    nc.vector.tensor_tensor(
                        st_ps[:], st_ps[:],
                        a_bc[:][0:SP, 475 * n:475 * (n + 1)], op=OP.mult)
                    nc.scalar.activation(
                        e10[:][:, 475 * n:475 * (n + 1)], st_ps[:],
                        ACTF.Exp, scale=snr_col[j][:])
                e20 = p_big.tile([SP, NF], BF16, tag=f"e20_{j}")
                nc.scalar.activation(e20[:], e10[:], ACTF.Square)
                E20.append(e20)
                cs = p_small.tile([SP, PP], F32, tag=f"cs{j}")
                nc.vector.tensor_reduce(
                    cs[:], e10[:].rearrange("p (q m) -> p q m", m=M),
                    axis=AX.X, op=OP.add)
                csr = p_small.tile([SP, PP], F32, tag=f"csr{j}")
                nc.vector.reciprocal_approx_fast(csr[:], cs[:])
                ft = p_big.tile([SP, NFF], BF16, tag=f"f_{j}")
                nc.vector.memset(
                    ft[:].rearrange("p (q r) -> p q r", r=FW)[:, :, M:FW], 1.0)
                nc.vector.tensor_tensor(
                    ft[:].rearrange("p (q r) -> p q r", r=FW)[:, :, 0:M],
                    e10[:].rearrange("p (q m) -> p q m", m=M),
                    csr[:].unsqueeze(2).broadcast_to((SP, PP, M)),
                    op=OP.mult)
                Ft.append(ft)

            # ---- phase 3b: FS row sums (ones-matmuls over F) + transpose ----
            fs_chunks = [(i * FW * 5, min(FW * 5, NFF - i * FW * 5)) for i in range(8)]
            fs_ps = [p_ps.tile([1, w], F32, tag="ps", name=f"fsps{_i}") for _i, (_, w) in enumerate(fs_chunks)]
            for j in range(STIL):
                for i, (off, w) in enumerate(fs_chunks):
                    nc.tensor.matmul(fs_ps[i][:], ones125b[:],
                                     Ft[j][:][:, off:off + w],
                                     start=(j == 0), stop=(j == STIL - 1))
            fs_row = p_rows.tile([1, NFF], F32, tag="rowbuf", name="rowfs")
            for i, (off, w) in enumerate(fs_chunks):
                nc.scalar.activation(fs_row[:][:, off:off + w], fs_ps[i][:], ACTF.Copy)
            nc.sync.dma_start(d_fs[:], fs_row[:])
            fs_col = p_small.tile([M, PP], F32)
            nc.sync.dma_start(
                fs_col[:],
                d_fs[:].rearrange("o (q r) -> r q", r=FW)[0:M, :])

            # ---- phase 4: per-pair Chat^T (+rs col) matmuls ----
            ctpad = p_big.tile([M, PP * CTW], BF16)
            nc.vector.memset(
                ctpad[:].rearrange("p (q r) -> p q r", r=CTW)[:, :, M:CTW], 0.0)
            rs_all = p_small.tile([M, PP], F32)
            for p in range(PP):
                ct_ps = p_ps.tile([M, FW], F32, tag="ps")
                for j in range(STIL):
                    nc.tensor.matmul(ct_ps[:],
                                     E20[j][:][:, M * p:M * (p + 1)],
                                     Ft[j][:][:, FW * p:FW * (p + 1)],
                                     start=(j == 0), stop=(j == STIL - 1))
                nc.vector.tensor_copy(ctpad[:][:, CTW * p:CTW * p + M], ct_ps[:][:, 0:M])
                nc.vector.tensor_copy(rs_all[:][:, p:p + 1], ct_ps[:][:, M:FW])


            # ---- phase 5: solve (Neumann in z space) ----
            rsr = p_small.tile([M, PP], F32)
            nc.vector.reciprocal_approx_fast(rsr[:], rs_all[:])
            qrsr = p_small.tile([M, PP], F32)
            nc.vector.tensor_scalar(qrsr[:], rsr[:], 0.25, None, op0=OP.mult)
            rhsv = p_small.tile([M, PP], F32)
            nc.vector.tensor_scalar(rhsv[:], fs_col[:], 0.5, 1.0, op0=OP.mult, op1=OP.add)
            qdr = p_small.tile([M, PP], F32)
            nc.vector.tensor_tensor(qdr[:], qrsr[:], rhsv[:], op=OP.mult)
            halffs = p_small.tile([M, PP], F32)
            nc.vector.tensor_scalar(halffs[:], fs_col[:], 0.5, None, op0=OP.mult)
            zt = p_small.tile([M, PP], BF16)
            nc.vector.tensor_copy(zt[:], qdr[:])
            kq_col = p_small.tile([M, PP], F32)
            wtmp = p_small.tile([M, PP], F32)
            for it in range(NITER):
                last = (it == NITER - 1)
                w_ps = p_ps.tile([CTW, PP], F32, tag="ps", name=f"w{it}")
                for p in range(PP):
                    nc.tensor.matmul(w_ps[:][:, p:p + 1],
                                     ctpad[:][:, CTW * p:CTW * (p + 1)],
                                     zt[:][:, p:p + 1], start=True, stop=True)
                if not last:
                    nc.vector.tensor_tensor(wtmp[:], w_ps[:][0:M, :], qrsr[:],
                                            op=OP.mult)
                    nc.vector.tensor_tensor(zt[:], wtmp[:], qdr[:], op=OP.add)
                else:
                    nc.vector.tensor_tensor(kq_col[:], w_ps[:][0:M, :], halffs[:],
                                            op=OP.add)

            # ---- phase 6: sums via PE ones-matvec, row-space normalize ----
            kq_bf = p_small.tile([M, PP], BF16)
            nc.vector.tensor_copy(kq_bf[:], kq_col[:])
            ssum_ps = p_ps.tile([PP, 1], F32, tag="ps", name="ssum")
            nc.tensor.matmul(ssum_ps[:], kq_bf[:], ones125b[:][0:M, :],
                             start=True, stop=True)
            sinv = p_small.tile([PP, 1], F32)
            nc.vector.reciprocal_approx_fast(sinv[:], ssum_ps[:])
            nc.sync.dma_start(d_kq[:].transpose([1, 0]), kq_col[:])
            kq_row = p_small.tile([PP, M], F32)
            nc.sync.dma_start(kq_row[:], d_kq[:])
            out_t = p_small.tile([PP, M], F32)
            nc.vector.tensor_scalar(out_t[:], kq_row[:], sinv[:], None, op0=OP.mult)
            nc.sync.dma_start(d_out[:], out_t[:])

    nc.compile()
    return nc


def shard_inputs(support_xf, query_xf):
    support_xf = np.asarray(support_xf, dtype=np.float32)
    query_xf = np.asarray(query_xf, dtype=np.float32)
    in_maps = []
    for core in range(8):
        b = core // 2
        half = core % 2
        qs = np.clip(np.arange(half * PP, half * PP + PP), 0, Q_ - 1)
        sup = np.ascontiguousarray(
            support_xf[b].reshape(NW, KS, C_, HW).transpose(2, 0, 3, 1).reshape(C_, S_ * HW))
        qry = np.ascontiguousarray(
            query_xf[b, qs].reshape(PP, C_, HW).transpose(1, 0, 2).reshape(C_, NF))
        in_maps.append({"sup": sup, "qry": qry})
    return in_maps


def run_sharded(support_xf, query_xf, trace=False, **kw):
    if "nc" not in _CACHED:
        _CACHED["nc"] = build_nc()
    nc = _CACHED["nc"]
    in_maps = shard_inputs(support_xf, query_xf)
    res = run_bass_kernel_spmd(nc, in_maps, core_ids=list(range(8)), trace=trace, **kw)
    b, q = np.asarray(support_xf).shape[0], np.asarray(query_xf).shape[1]
    out = np.zeros((b, q, 1, 10, 10), np.float32)
    for core in range(8):
        bi = core // 2
        half = core % 2
        real = min(PP, q - half * PP)
        o = res.results[core]["out"][:real]
        out[bi, half * PP: half * PP + real] = o.reshape(real, 1, 10, 10)
    return out, res


def kernel(support_xf, query_xf, n_way=5, k_shot=5):
    out, _ = run_sharded(support_xf, query_xf, trace=False)
    return out
